# revision 3
# baseline (speedup 1.0000x reference)
"""Trainium2 Bass kernel for per-edge dot products (u_dot_v / DotPredictor).

score[e] = dot(h[src[e]], h[dst[e]]) with h: [50000, 128] f32, src/dst: [640000] i64.

Strategy (8 NeuronCores):
  - Shard edges contiguously: 80000 edges per core. The node table h is
    replicated to every core (it stays in HBM; rows are fetched on demand).
  - Per core, both row streams (h[src], h[dst]) are fetched with the SWDGE
    `dma_gather` custom DMA instruction (512B rows from HBM into SBUF,
    edge-major layout [128, chunk, 128]).
  - `dma_gather` indices are int16, so the table is split into two halves of
    25000 rows and each core's edges are bucketed into 4 groups by
    (src_half, dst_half). Group sizes are padded to a shared per-group cap
    (max over cores, rounded to 128) so all cores run one SPMD program.
  - The per-edge dot is a fused custom DVE op (affine_mul_reduce): computes
    (hu * hv) and the 128-wide row sum in a single pass per 128-edge chunk,
    writing the product in-place over the gathered hu tile (no extra SBUF
    traffic, no same-address WAW hazards).
  - Host side: bucketing/permutation of edges (sorted by src within each
    group for HBM locality), int16 index wrapping ([128, N/16] layout the
    Q7 gather ucode expects), and inverse permutation of the returned
    scores.

  Measured on TRN2 (8 cores, trace core 0): ~393 us HW exec. The kernel is
  bound by SWDGE gather descriptor drain: each 512B-row descriptor costs
  ~31ns on its SDMA engine, plus a ~1.4us ring-refill bubble per 4-gather
  cycle (each SWDGE queue ring only fits one 1024-index gather; the ring
  carveout size is fixed by the runtime).
"""

import sys

sys.path.insert(0, "/opt/trn_rl_repo")

from contextlib import ExitStack

import numpy as np

import concourse.bacc as bacc
import concourse.bass as bass
import concourse.mybir as mybir
from concourse import library_config
from concourse.bass_utils import run_bass_kernel_spmd

N_NODES = 50000
D = 128
HALF = 25000
M = 8  # cores
# SWDGE ring carveout is ~64-65 descriptors per engine per queue (a single
# dma_gather may carry at most 1024 indices / 65 descs per engine before the
# ucode hits illegal_instruction in reclaim_for). A 1024-idx gather fills a
# whole ring, so the 4 queues drain in lockstep and every ring goes empty for
# ~3us while the sem chain (gather sems -> vector -> c_sem -> gpsimd issue)
# restarts — measured 60% SDMA-engine occupancy. With 384-idx gathers
# (24+1=25 descs/engine) TWO gathers fit per ring, so the next gather's
# descriptors are already ring-resident when the previous one finishes and
# the engines never starve.
TILE = 384  # max gathered edges per DMA tile (per stream)
NQ = 4  # SWDGE queues
NBUF = 16  # gather buffer slots per stream

# group order chosen so consecutive groups share a table half where possible
GROUP_SRCS = [(0, 0), (0, 1), (1, 1), (1, 0)]  # (src_half, dst_half) per group

_cache = {}

# test harness hooks: set TRACE=True before calling kernel() to profile;
# the BassKernelResults of the last run lands in LAST_RESULTS.
TRACE = False
LAST_RESULTS = None


def _build(caps):
    """Build (and cache) the SPMD bass program for the given per-group caps."""
    key = tuple(caps)
    if key in _cache:
        return _cache[key]

    npad = int(sum(caps))
    nch = npad // 128
    ncol = npad // 16

    # tile list: split each group segment into <=TILE pieces (multiples of
    # 128).
    bounds = np.cumsum([0] + list(caps))
    tiles = []  # (start, length, src_half_u, src_half_v)
    for g in range(4):
        su, sv = GROUP_SRCS[g]
        p = int(bounds[g])
        while p < bounds[g + 1]:
            ln = int(min(TILE, bounds[g + 1] - p))
            tiles.append((p, ln, su, sv))
            p += ln
    T = len(tiles)

    nc = bacc.Bacc("TRN2", debug=False, num_swdge_queues=NQ)
    h0 = nc.dram_tensor("h0", [HALF, D], mybir.dt.float32, kind="ExternalInput")
    h1 = nc.dram_tensor("h1", [HALF, D], mybir.dt.float32, kind="ExternalInput")
    iu = nc.dram_tensor("iu", [128, ncol], mybir.dt.int16, kind="ExternalInput")
    iv = nc.dram_tensor("iv", [128, ncol], mybir.dt.int16, kind="ExternalInput")
    score = nc.dram_tensor("score", [128, nch], mybir.dt.float32, kind="ExternalOutput")
    halves = {0: h0, 1: h1}

    with (
        nc.sbuf_tensor("iu_sb", [128, ncol], mybir.dt.int16) as iu_sb,
        nc.sbuf_tensor("iv_sb", [128, ncol], mybir.dt.int16) as iv_sb,
        nc.sbuf_tensor("hu_sb", [128, NBUF, TILE // 128, D], mybir.dt.float32) as hu_sb,
        nc.sbuf_tensor("hv_sb", [128, NBUF, TILE // 128, D], mybir.dt.float32) as hv_sb,
        nc.sbuf_tensor("score_sb", [128, nch], mybir.dt.float32) as score_sb,
        nc.semaphore("iu_h_sem") as iu_h_sem,
        nc.semaphore("iv_h_sem") as iv_h_sem,
        nc.semaphore("rest_sem") as rest_sem,
        nc.semaphore("c_sem") as c_sem,
        nc.semaphore("o_sem") as o_sem,
        ExitStack() as _stack,
        nc.Block() as block,
    ):
        # A DMA .then_inc(sem, 16) lands as 16 independent +1s (one per SDMA
        # engine), so in-flight gathers must not share a semaphore: rotate
        # per buffer slot. Reuse after NBUF tiles is safe because the gpsimd
        # c_sem wait guarantees tile t-NBUF's gather fully completed (its
        # data was consumed) before tile t's gather is issued.
        gu_sems = [_stack.enter_context(nc.semaphore(f"gu_sem{i}")) for i in range(NBUF)]
        gv_sems = [_stack.enter_context(nc.semaphore(f"gv_sem{i}")) for i in range(NBUF)]

        # idx loads are split: a small head (first 2 tiles) lets gathers
        # start ~10us earlier; the bulk arrives while tiles 0-1 drain.
        hc = min(2 * TILE // 16, ncol)

        @block.sync
        def _(sync):
            sync.dma_start(iu_sb[:, :hc], iu[:, :hc]).then_inc(iu_h_sem, 16)
            sync.dma_start(iv_sb[:, :hc], iv[:, :hc]).then_inc(iv_h_sem, 16)
            if hc < ncol:
                sync.dma_start(iu_sb[:, hc:], iu[:, hc:]).then_inc(rest_sem, 16)
                sync.dma_start(iv_sb[:, hc:], iv[:, hc:]).then_inc(rest_sem, 16)
            sync.wait_ge(c_sem, T)
            sync.dma_start(score[:], score_sb[:]).then_inc(o_sem, 16)
            sync.wait_ge(o_sem, 16)

        @block.gpsimd
        def _(gp):
            gp.load_library(library_config.mlp)
            gp.wait_ge(iu_h_sem, 16)
            gp.wait_ge(iv_h_sem, 16)
            rest_waited = hc >= ncol
            for t, (p, ln, su, sv) in enumerate(tiles):
                if not rest_waited and (p + ln) // 16 > hc:
                    gp.wait_ge(rest_sem, 32)
                    rest_waited = True
                slot = t % NBUF
                if t >= NBUF:
                    # buffer slot reusable once compute of tile t-NBUF is done
                    gp.wait_ge(c_sem, t - NBUF + 1)
                gp.dma_gather(
                    hu_sb[:, slot, : ln // 128, :],
                    halves[su][:],
                    iu_sb[:, p // 16 : (p + ln) // 16],
                    ln,
                    ln,
                    D,
                    queue_num=(2 * t) % NQ,
                    single_packet=False,
                ).then_inc(gu_sems[slot], 16)
                gp.dma_gather(
                    hv_sb[:, slot, : ln // 128, :],
                    halves[sv][:],
                    iv_sb[:, p // 16 : (p + ln) // 16],
                    ln,
                    ln,
                    D,
                    queue_num=(2 * t + 1) % NQ,
                    single_packet=False,
                ).then_inc(gv_sems[slot], 16)

        @block.vector
        def _(vec):
            for t, (p, ln, su, sv) in enumerate(tiles):
                slot = t % NBUF
                k = t // NBUF + 1
                vec.wait_ge(gu_sems[slot], 16 * k)
                vec.wait_ge(gv_sems[slot], 16 * k)
                last = None
                base = p // 128
                for c in range(ln // 128):
                    last = vec.affine_mul_reduce(
                        out=hu_sb[:, slot, c, :],
                        accum_out=score_sb[:, base + c : base + c + 1],
                        in0=hu_sb[:, slot, c, :],
                        in1=hv_sb[:, slot, c, :],
                        scale=1.0,
                        bias=0.0,
                    )
                last.then_inc(c_sem, 1)

    nc.finalize()
    _cache[key] = (nc, npad)
    return nc, npad


def _wrap_idx(vec):
    """int16 idx vector [NPAD] -> [128, NPAD/16] SWDGE layout.

    idx j lives at partition j%16, column j//16; the 16-partition block is
    replicated 8x so each Q7 core sees it in its own partition group."""
    blk = vec.reshape(-1, 16).T
    return np.ascontiguousarray(np.tile(blk, (8, 1)), dtype=np.int16)


def kernel(h=None, src=None, dst=None):
    h = np.ascontiguousarray(np.asarray(h, dtype=np.float32))
    src = np.asarray(src).astype(np.int64)
    dst = np.asarray(dst).astype(np.int64)
    E = src.shape[0]
    assert E % M == 0
    ec = E // M

    src_sh = src.reshape(M, ec)
    dst_sh = dst.reshape(M, ec)

    orders, all_counts = [], []
    for m in range(M):
        gs = (src_sh[m] >= HALF).astype(np.int64)
        gd = (dst_sh[m] >= HALF).astype(np.int64)
        gid = 2 * gs + (gs ^ gd)  # maps (0,0)->0 (0,1)->1 (1,1)->2 (1,0)->3
        # sort by src within each group: the hu gather then reads the table
        # in (nearly) ascending address order, improving HBM row locality
        order = np.lexsort((src_sh[m], gid))
        counts = np.bincount(gid, minlength=4)
        orders.append(order)
        all_counts.append(counts)
    all_counts = np.stack(all_counts)  # [M, 4]
    caps = [int(-(-int(all_counts[:, g].max()) // 128) * 128) for g in range(4)]
    caps = [max(c, 128) for c in caps]

    nc, npad = _build(caps)
    bounds = np.cumsum([0] + list(caps))

    in_maps = []
    h0 = np.ascontiguousarray(h[:HALF])
    h1 = np.ascontiguousarray(h[HALF:])
    for m in range(M):
        iu_pad = np.zeros(npad, np.int16)
        iv_pad = np.zeros(npad, np.int16)
        order, counts = orders[m], all_counts[m]
        prefix = np.cumsum(np.concatenate([[0], counts]))
        for g in range(4):
            su, sv = GROUP_SRCS[g]
            idxs = order[prefix[g] : prefix[g + 1]]
            n = len(idxs)
            b = int(bounds[g])
            iu_pad[b : b + n] = (src_sh[m][idxs] - HALF * su).astype(np.int16)
            iv_pad[b : b + n] = (dst_sh[m][idxs] - HALF * sv).astype(np.int16)
        in_maps.append(
            {"h0": h0, "h1": h1, "iu": _wrap_idx(iu_pad), "iv": _wrap_idx(iv_pad)}
        )

    res = run_bass_kernel_spmd(nc, in_maps, core_ids=list(range(M)), trace=TRACE)
    global LAST_RESULTS
    LAST_RESULTS = res

    out = np.empty(E, np.float32)
    for m in range(M):
        vec = res.results[m]["score"].T.reshape(-1)  # padded pos = c*128+p
        order, counts = orders[m], all_counts[m]
        prefix = np.cumsum(np.concatenate([[0], counts]))
        for g in range(4):
            n = int(counts[g])
            b = int(bounds[g])
            out[m * ec + order[prefix[g] : prefix[g] + n]] = vec[b : b + n]
    return out



# revision 5
# speedup vs baseline: 1.3473x; 1.3473x over previous
"""Trainium2 Bass kernel for per-edge dot products (u_dot_v / DotPredictor).

score[e] = dot(h[src[e]], h[dst[e]]) with h: [50000, 128] f32, src/dst: [640000] i64.

Strategy (8 NeuronCores):
  - Shard edges contiguously: 80000 edges per core. The node table h is
    replicated to every core (it stays in HBM; rows are fetched on demand).
  - Per core, both row streams (h[src], h[dst]) are fetched with the SWDGE
    `dma_gather` custom DMA instruction (512B rows from HBM into SBUF,
    edge-major layout [128, chunk, 128]).
  - `dma_gather` indices are int16, so the table is split into two halves of
    25000 rows and each core's edges are bucketed into 4 groups by
    (src_half, dst_half). Group sizes are padded to a shared per-group cap
    (max over cores, rounded to 128) so all cores run one SPMD program.
  - The per-edge dot is a fused custom DVE op (affine_mul_reduce): computes
    (hu * hv) and the 128-wide row sum in a single pass per 128-edge chunk,
    writing the product in-place over the gathered hu tile (no extra SBUF
    traffic, no same-address WAW hazards).
  - Host side: bucketing/permutation of edges (sorted by src within each
    group for HBM locality), int16 index wrapping ([128, N/16] layout the
    Q7 gather ucode expects), and inverse permutation of the returned
    scores.

  Measured on TRN2 (8 cores, trace core 0): ~393 us HW exec. The kernel is
  bound by SWDGE gather descriptor drain: each 512B-row descriptor costs
  ~31ns on its SDMA engine, plus a ~1.4us ring-refill bubble per 4-gather
  cycle (each SWDGE queue ring only fits one 1024-index gather; the ring
  carveout size is fixed by the runtime).
"""

import sys

sys.path.insert(0, "/opt/trn_rl_repo")

from contextlib import ExitStack

import numpy as np

import concourse.bacc as bacc
import concourse.bass as bass
import concourse.mybir as mybir
from concourse import library_config
from concourse.bass_utils import run_bass_kernel_spmd

N_NODES = 50000
D = 128
HALF = 25000
M = 8  # cores
# SWDGE ring carveout is ~64-65 descriptors per engine per queue (a single
# dma_gather may carry at most 1024 indices / 65 descs per engine before the
# ucode hits illegal_instruction in reclaim_for). A 1024-idx gather fills a
# whole ring, so the 4 queues drain in lockstep and every ring goes empty for
# ~3us while the sem chain (gather sems -> vector -> c_sem -> gpsimd issue)
# restarts — measured 60% SDMA-engine occupancy. With 384-idx gathers
# (24+1=25 descs/engine) TWO gathers fit per ring, so the next gather's
# descriptors are already ring-resident when the previous one finishes and
# the engines never starve.
TILE = 1024  # max gathered edges per DMA tile (per stream)
NQ = 4  # SWDGE queues
NBUF = 8  # gather buffer slots per stream
DMA_SCRATCH = 65536  # SBUF descriptor-ring carveout bytes (default 16384)

# group order chosen so consecutive groups share a table half where possible
GROUP_SRCS = [(0, 0), (0, 1), (1, 1), (1, 0)]  # (src_half, dst_half) per group

_cache = {}

# test harness hooks: set TRACE=True before calling kernel() to profile;
# the BassKernelResults of the last run lands in LAST_RESULTS.
TRACE = False
LAST_RESULTS = None


def _build(caps):
    """Build (and cache) the SPMD bass program for the given per-group caps."""
    key = tuple(caps)
    if key in _cache:
        return _cache[key]

    npad = int(sum(caps))
    nch = npad // 128
    ncol = npad // 16

    # tile list: split each group segment into <=TILE pieces (multiples of
    # 128).
    bounds = np.cumsum([0] + list(caps))
    tiles = []  # (start, length, src_half_u, src_half_v)
    for g in range(4):
        su, sv = GROUP_SRCS[g]
        p = int(bounds[g])
        while p < bounds[g + 1]:
            ln = int(min(TILE, bounds[g + 1] - p))
            tiles.append((p, ln, su, sv))
            p += ln
    T = len(tiles)

    nc = bacc.Bacc(
        "TRN2",
        debug=False,
        num_swdge_queues=NQ,
        dynamic_dma_scratch_size=DMA_SCRATCH,
    )
    h0 = nc.dram_tensor("h0", [HALF, D], mybir.dt.float32, kind="ExternalInput")
    h1 = nc.dram_tensor("h1", [HALF, D], mybir.dt.float32, kind="ExternalInput")
    iu = nc.dram_tensor("iu", [128, ncol], mybir.dt.int16, kind="ExternalInput")
    iv = nc.dram_tensor("iv", [128, ncol], mybir.dt.int16, kind="ExternalInput")
    score = nc.dram_tensor("score", [128, nch], mybir.dt.float32, kind="ExternalOutput")
    halves = {0: h0, 1: h1}

    with (
        nc.sbuf_tensor("iu_sb", [128, ncol], mybir.dt.int16) as iu_sb,
        nc.sbuf_tensor("iv_sb", [128, ncol], mybir.dt.int16) as iv_sb,
        nc.sbuf_tensor("hu_sb", [128, NBUF, TILE // 128, D], mybir.dt.float32) as hu_sb,
        nc.sbuf_tensor("hv_sb", [128, NBUF, TILE // 128, D], mybir.dt.float32) as hv_sb,
        nc.sbuf_tensor("score_sb", [128, nch], mybir.dt.float32) as score_sb,
        nc.semaphore("iu_h_sem") as iu_h_sem,
        nc.semaphore("iv_h_sem") as iv_h_sem,
        nc.semaphore("rest_sem") as rest_sem,
        nc.semaphore("c_sem") as c_sem,
        nc.semaphore("o_sem") as o_sem,
        ExitStack() as _stack,
        nc.Block() as block,
    ):
        # A DMA .then_inc(sem, 16) lands as 16 independent +1s (one per SDMA
        # engine), so in-flight gathers must not share a semaphore: rotate
        # per buffer slot. Reuse after NBUF tiles is safe because the gpsimd
        # c_sem wait guarantees tile t-NBUF's gather fully completed (its
        # data was consumed) before tile t's gather is issued.
        gu_sems = [_stack.enter_context(nc.semaphore(f"gu_sem{i}")) for i in range(NBUF)]
        gv_sems = [_stack.enter_context(nc.semaphore(f"gv_sem{i}")) for i in range(NBUF)]

        # idx loads are split: a small head (first 2 tiles) lets gathers
        # start ~10us earlier; the bulk arrives while tiles 0-1 drain.
        hc = min(2 * TILE // 16, ncol)

        @block.sync
        def _(sync):
            sync.dma_start(iu_sb[:, :hc], iu[:, :hc]).then_inc(iu_h_sem, 16)
            sync.dma_start(iv_sb[:, :hc], iv[:, :hc]).then_inc(iv_h_sem, 16)
            if hc < ncol:
                sync.dma_start(iu_sb[:, hc:], iu[:, hc:]).then_inc(rest_sem, 16)
                sync.dma_start(iv_sb[:, hc:], iv[:, hc:]).then_inc(rest_sem, 16)
            sync.wait_ge(c_sem, T)
            sync.dma_start(score[:], score_sb[:]).then_inc(o_sem, 16)
            sync.wait_ge(o_sem, 16)

        @block.gpsimd
        def _(gp):
            gp.load_library(library_config.mlp)
            gp.wait_ge(iu_h_sem, 16)
            gp.wait_ge(iv_h_sem, 16)
            rest_waited = hc >= ncol
            for t, (p, ln, su, sv) in enumerate(tiles):
                if not rest_waited and (p + ln) // 16 > hc:
                    gp.wait_ge(rest_sem, 32)
                    rest_waited = True
                slot = t % NBUF
                if t >= NBUF:
                    # buffer slot reusable once compute of tile t-NBUF is done
                    gp.wait_ge(c_sem, t - NBUF + 1)
                gp.dma_gather(
                    hu_sb[:, slot, : ln // 128, :],
                    halves[su][:],
                    iu_sb[:, p // 16 : (p + ln) // 16],
                    ln,
                    ln,
                    D,
                    queue_num=(2 * t) % NQ,
                    single_packet=False,
                ).then_inc(gu_sems[slot], 16)
                gp.dma_gather(
                    hv_sb[:, slot, : ln // 128, :],
                    halves[sv][:],
                    iv_sb[:, p // 16 : (p + ln) // 16],
                    ln,
                    ln,
                    D,
                    queue_num=(2 * t + 1) % NQ,
                    single_packet=False,
                ).then_inc(gv_sems[slot], 16)

        @block.vector
        def _(vec):
            for t, (p, ln, su, sv) in enumerate(tiles):
                slot = t % NBUF
                k = t // NBUF + 1
                vec.wait_ge(gu_sems[slot], 16 * k)
                vec.wait_ge(gv_sems[slot], 16 * k)
                last = None
                base = p // 128
                for c in range(ln // 128):
                    last = vec.affine_mul_reduce(
                        out=hu_sb[:, slot, c, :],
                        accum_out=score_sb[:, base + c : base + c + 1],
                        in0=hu_sb[:, slot, c, :],
                        in1=hv_sb[:, slot, c, :],
                        scale=1.0,
                        bias=0.0,
                    )
                last.then_inc(c_sem, 1)

    nc.finalize()
    _cache[key] = (nc, npad)
    return nc, npad


def _wrap_idx(vec):
    """int16 idx vector [NPAD] -> [128, NPAD/16] SWDGE layout.

    idx j lives at partition j%16, column j//16; the 16-partition block is
    replicated 8x so each Q7 core sees it in its own partition group."""
    blk = vec.reshape(-1, 16).T
    return np.ascontiguousarray(np.tile(blk, (8, 1)), dtype=np.int16)


def kernel(h=None, src=None, dst=None):
    h = np.ascontiguousarray(np.asarray(h, dtype=np.float32))
    src = np.asarray(src).astype(np.int64)
    dst = np.asarray(dst).astype(np.int64)
    E = src.shape[0]
    assert E % M == 0
    ec = E // M

    src_sh = src.reshape(M, ec)
    dst_sh = dst.reshape(M, ec)

    orders, all_counts = [], []
    for m in range(M):
        gs = (src_sh[m] >= HALF).astype(np.int64)
        gd = (dst_sh[m] >= HALF).astype(np.int64)
        gid = 2 * gs + (gs ^ gd)  # maps (0,0)->0 (0,1)->1 (1,1)->2 (1,0)->3
        # sort by src within each group: the hu gather then reads the table
        # in (nearly) ascending address order, improving HBM row locality
        order = np.lexsort((src_sh[m], gid))
        counts = np.bincount(gid, minlength=4)
        orders.append(order)
        all_counts.append(counts)
    all_counts = np.stack(all_counts)  # [M, 4]
    caps = [int(-(-int(all_counts[:, g].max()) // 128) * 128) for g in range(4)]
    caps = [max(c, 128) for c in caps]

    nc, npad = _build(caps)
    bounds = np.cumsum([0] + list(caps))

    in_maps = []
    h0 = np.ascontiguousarray(h[:HALF])
    h1 = np.ascontiguousarray(h[HALF:])
    for m in range(M):
        iu_pad = np.zeros(npad, np.int16)
        iv_pad = np.zeros(npad, np.int16)
        order, counts = orders[m], all_counts[m]
        prefix = np.cumsum(np.concatenate([[0], counts]))
        for g in range(4):
            su, sv = GROUP_SRCS[g]
            idxs = order[prefix[g] : prefix[g + 1]]
            n = len(idxs)
            b = int(bounds[g])
            iu_pad[b : b + n] = (src_sh[m][idxs] - HALF * su).astype(np.int16)
            iv_pad[b : b + n] = (dst_sh[m][idxs] - HALF * sv).astype(np.int16)
        in_maps.append(
            {"h0": h0, "h1": h1, "iu": _wrap_idx(iu_pad), "iv": _wrap_idx(iv_pad)}
        )

    res = run_bass_kernel_spmd(nc, in_maps, core_ids=list(range(M)), trace=TRACE)
    global LAST_RESULTS
    LAST_RESULTS = res

    out = np.empty(E, np.float32)
    for m in range(M):
        vec = res.results[m]["score"].T.reshape(-1)  # padded pos = c*128+p
        order, counts = orders[m], all_counts[m]
        prefix = np.cumsum(np.concatenate([[0], counts]))
        for g in range(4):
            n = int(counts[g])
            b = int(bounds[g])
            out[m * ec + order[prefix[g] : prefix[g] + n]] = vec[b : b + n]
    return out

